# revision 17
# baseline (speedup 1.0000x reference)
"""Deformable transformer encoder layer on 8 Trainium2 NeuronCores.

Sharding: core c handles batch b=c//4, query rows [32j, 32j+32), j=c%4.
Each core builds full-image per-head bilinear gather tables (dual-copy,
256B elements), samples with SWDGE dma_gather, then out-proj + LN + FFN
+ LN for its 4096 queries. Host only splits inputs / concats outputs.
"""
import sys, os
sys.path.insert(0, '/opt/trn_rl_repo')
SKIP_GATHER = bool(int(os.environ.get("K_SKIP_GATHER", "0")))
IDX0 = bool(int(os.environ.get("K_IDX0", "0")))
import numpy as np

import concourse.bass as bass
import concourse.bacc as bacc
import concourse.mybir as mybir
import concourse.tile as tile
from concourse import masks
from concourse.bass_utils import run_bass_kernel_spmd

F32 = mybir.dt.float32
F32R = mybir.dt.float32r
I32 = mybir.dt.int32
I16 = mybir.dt.int16
AL = mybir.AluOpType
AF = mybir.ActivationFunctionType
AX = mybir.AxisListType

D, NH, DH, NP, DFF = 256, 8, 32, 4, 2048
B, H, W = 2, 128, 128
HP, WP = H + 4, W + 4            # zero-padded image dims
NPAD = HP * WP                   # 17424 padded tokens
NE = NPAD // 2                   # 8712 elements per table copy
QROWS = 32                       # query rows per core
NG = 8                           # groups of 4 query tiles

_nc_cache = None


def build_nc():
    nc = bacc.Bacc(None, target_bir_lowering=False)

    src_img = nc.dram_tensor("src_img", [H * W, D], F32R, kind="ExternalInput")
    src_q = nc.dram_tensor("src_q", [QROWS * W, D], F32R, kind="ExternalInput")
    rowbase = nc.dram_tensor("rowbase", [1, 1], F32, kind="ExternalInput")
    w_off = nc.dram_tensor("w_off", [NH * NP * 2, D], F32R, kind="ExternalInput")
    b_off = nc.dram_tensor("b_off", [1, NH * NP * 2], F32, kind="ExternalInput")
    w_attn = nc.dram_tensor("w_attn", [NH * NP, D], F32R, kind="ExternalInput")
    b_attn = nc.dram_tensor("b_attn", [1, NH * NP], F32, kind="ExternalInput")
    w_val = nc.dram_tensor("w_val", [D, D], F32R, kind="ExternalInput")
    b_val = nc.dram_tensor("b_val", [1, D], F32, kind="ExternalInput")
    w_out = nc.dram_tensor("w_out", [D, D], F32R, kind="ExternalInput")
    b_out = nc.dram_tensor("b_out", [1, D], F32, kind="ExternalInput")
    w1 = nc.dram_tensor("w1", [DFF, D], F32R, kind="ExternalInput")
    b1 = nc.dram_tensor("b1", [1, DFF], F32, kind="ExternalInput")
    w2 = nc.dram_tensor("w2", [D, DFF], F32R, kind="ExternalInput")
    b2 = nc.dram_tensor("b2", [1, D], F32, kind="ExternalInput")
    g1 = nc.dram_tensor("g1", [1, D], F32, kind="ExternalInput")
    be1 = nc.dram_tensor("be1", [1, D], F32, kind="ExternalInput")
    g2 = nc.dram_tensor("g2", [1, D], F32, kind="ExternalInput")
    be2 = nc.dram_tensor("be2", [1, D], F32, kind="ExternalInput")
    out = nc.dram_tensor("out", [QROWS * W, D], F32, kind="ExternalOutput")
    # per-head dual-copy gather tables, elements of 64 f32 (256B)
    tabs = [nc.dram_tensor(f"tabs{h}", [2 * NE, 64], F32) for h in range(NH)]
    tabf = [t[:].rearrange("e c -> (e c)") for t in tabs]  # flat f32 units

    def T0(y):  # first padded token of image row y
        return (y + 2) * WP + 2

    with tile.TileContext(nc) as tc:
      with (
        tc.tile_pool(name="const", bufs=1) as cpool,
        tc.tile_pool(name="wpool", bufs=1) as wpool,
        tc.tile_pool(name="ps_sh", bufs=3, space="PSUM") as ps_sh,
      ):
        # ---------------- constants ----------------
        ident_f = cpool.tile([128, 128], F32)
        masks.make_identity(nc, ident_f[:])
        ident = cpool.tile([128, 128], F32R)
        nc.vector.tensor_copy(ident[:], ident_f[:])

        sels = []
        for m in range(8):
            s = cpool.tile([128, 128], F32, tag=f"sel{m}")
            nc.gpsimd.memset(s[:], 0.0)
            nc.gpsimd.affine_select(
                out=s[:], in_=s[:], compare_op=AL.not_equal, fill=1.0,
                base=-16 * m, pattern=[[0, 8], [-1, 16]], channel_multiplier=1)
            sels.append(s)

        ones_r = cpool.tile([1, 128], F32)
        nc.vector.memset(ones_r[:], 1.0)

        iox = cpool.tile([128, 1], I32)
        nc.gpsimd.iota(iox[:], pattern=[[1, 1]], base=0, channel_multiplier=1)
        ix = cpool.tile([128, 1], F32)
        nc.vector.tensor_copy(ix[:], iox[:])

        def bcast_cols(dram_row, n, name):
            """[1, n] DRAM -> [128, n] SBUF (outer product with ones)."""
            r = cpool.tile([1, n], F32, tag=f"ob_{name}")
            nc.sync.dma_start(r[:], dram_row)
            ps = ps_sh.tile([128, n], F32, tag="s")
            nc.tensor.matmul(ps[:], ones_r[:], r[:])
            t = cpool.tile([128, n], F32, tag=f"obf_{name}")
            nc.vector.tensor_copy(t[:], ps[:])
            return t

        rb = bcast_cols(rowbase[:], 1, "rb")          # [128,1] = 32*j
        boa_full = cpool.tile([128, 96], F32)
        nc.vector.tensor_copy(boa_full[:, 0:64], bcast_cols(b_off[:], 64, "boff")[:])
        nc.vector.tensor_copy(boa_full[:, 64:96], bcast_cols(b_attn[:], 32, "batt")[:])
        bval_full = bcast_cols(b_val[:], D, "bval")
        bout_full = bcast_cols(b_out[:], D, "bout")
        b2_full = bcast_cols(b2[:], D, "b2")
        g1_full = bcast_cols(g1[:], D, "g1")
        be1_full = bcast_cols(be1[:], D, "be1")
        g2_full = bcast_cols(g2[:], D, "g2")
        be2_full = bcast_cols(be2[:], D, "be2")

        # b1 in ff1T layout: b1t[p, dfc] = b1[dfc*128 + p]
        b1t = cpool.tile([128, 16], F32)
        nc.sync.dma_start(
            b1t[:], b1[:].rearrange("a (hi p) -> a hi p", p=128)[0].transpose([1, 0]))

        # ---------------- weight transposes ----------------
        wload_cm = tc.tile_pool(name="wload", bufs=1)
        wload = wload_cm.__enter__()

        def load_transpose(wdram, rows, name):
            """[rows=256, 256] DRAM -> 2 f32r tiles [128 (d chunk), 256] = W.T"""
            assert rows == 256
            outs = [wpool.tile([128, rows], F32R, tag=f"ltT_{name}{k}", name=f"ltT_{name}{k}")
                    for k in range(2)]
            for rc in range(2):
                wsb = wload.tile([128, D], F32R, tag=f"lt_{name}{rc}")
                nc.sync.dma_start(wsb[:], wdram[rc * 128:(rc + 1) * 128, :])
                for k in range(2):
                    ps = ps_sh.tile([128, 128], F32R, tag="s")
                    nc.tensor.matmul(ps[:], wsb[:, k * 128:(k + 1) * 128],
                                     ident[:], is_transpose=True)
                    nc.vector.tensor_copy(
                        outs[k][:, rc * 128:(rc + 1) * 128], ps[:])
            return outs

        WvT = load_transpose(w_val[:], D, "wv")
        WoT = load_transpose(w_out[:], D, "wo")
        woa = wload.tile([96, D], F32R)
        nc.sync.dma_start(woa[0:64, :], w_off[:])
        nc.sync.dma_start(woa[64:96, :], w_attn[:])
        WoaT = []
        for k in range(2):
            ps = ps_sh.tile([128, 96], F32R, tag="s")
            nc.tensor.matmul(ps[:, 0:96], woa[:, k * 128:(k + 1) * 128],
                             ident[0:96, 0:96], is_transpose=True)
            t = wpool.tile([128, 96], F32R, tag=f"woaT{k}")
            nc.vector.tensor_copy(t[:], ps[:, 0:96])
            WoaT.append(t)

        W1T = [[None] * 16 for _ in range(2)]   # [ck][dfc]: [128 c, 128 dff]
        for dfc in range(16):
            wsb = wload.tile([128, D], F32R, tag="w1_ld")
            nc.sync.dma_start(wsb[:], w1[dfc * 128:(dfc + 1) * 128, :])
            for ck in range(2):
                ps = ps_sh.tile([128, 128], F32R, tag="s")
                nc.tensor.matmul(ps[:], wsb[:, ck * 128:(ck + 1) * 128],
                                 ident[:], is_transpose=True)
                t = wpool.tile([128, 128], F32R, tag=f"w1T_{ck}_{dfc}")
                nc.vector.tensor_copy(t[:], ps[:])
                W1T[ck][dfc] = t

        W2T = []                                 # [dfc]: [128 dff, 256 c]
        w2sb = []
        for ck in range(2):
            wsb = wload.tile([128, DFF], F32R, tag=f"w2_ld{ck}")
            nc.sync.dma_start(wsb[:], w2[ck * 128:(ck + 1) * 128, :])
            w2sb.append(wsb)
        for dfc in range(16):
            t = wpool.tile([128, D], F32R, tag=f"w2T_{dfc}")
            for ck in range(2):
                ps = ps_sh.tile([128, 128], F32R, tag="s")
                nc.tensor.matmul(ps[:], w2sb[ck][:, dfc * 128:(dfc + 1) * 128],
                                 ident[:], is_transpose=True)
                nc.vector.tensor_copy(t[:, ck * 128:(ck + 1) * 128], ps[:])
            W2T.append(t)

        wload_cm.__exit__(None, None, None)
        phase_cm = []
        def _open(name, **kw):
            cm = tc.tile_pool(name=name, **kw)
            phase_cm.append(cm)
            return cm.__enter__()
        p1 = _open("p1", bufs=3)
        p2 = _open("p2", bufs=2)
        gp = _open("gp", bufs=3)
        ps_f = _open("ps_f", bufs=2, space="PSUM")
        ps_o = _open("ps_o", bufs=2, space="PSUM")

        # ---------------- zero-fill table pads ----------------
        zt = cpool.tile([128, 128], F32)
        nc.vector.memset(zt[:], 0.0)

        def zero_fill(h, cp, tok0, ntok):
            """write ntok*32 zero f32 at token tok0 of table copy cp."""
            off = cp * (NE * 64) + (tok0 - cp) * 32
            left = ntok
            while left > 0:
                a = min(128, left)
                nc.sync.dma_start(
                    tabf[h][off:off + a * 32].rearrange("(a b) -> a b", b=32),
                    zt[0:a, 0:32])
                off += a * 32
                left -= a

        for h in range(NH):
            for cp in range(2):
                zero_fill(h, cp, cp, T0(0) - cp)                 # leading
                zero_fill(h, cp, T0(127) + W, NPAD + cp - (T0(127) + W))  # trailing
                # row gaps: 127 runs of 4 tokens, stride 132 tokens
                gap0 = cp * (NE * 64) + (T0(0) + W - cp) * 32
                nc.sync.dma_start(
                    tabf[h][gap0:gap0 + 127 * WP * 32]
                    .rearrange("(y r) -> y r", r=WP * 32)[:, 0:128],
                    zt[0:127, 0:128])

        # ---------------- phase 1: value + tables ----------------
        for batch in range(16):
            vbat = p1.tile([128, 8 * D], F32, tag="vbat", bufs=2)
            for yb in range(8):
                y = batch * 8 + yb
                xr = p1.tile([128, D], F32R, tag="p1_x")
                nc.sync.dma_start(xr[:], src_img[y * W:(y + 1) * W, :])
                xT = p1.tile([128, D], F32R, tag="p1_xT")
                for k in range(2):
                    ps = ps_sh.tile([128, 128], F32R, tag="s")
                    nc.tensor.matmul(ps[:], xr[:, k * 128:(k + 1) * 128],
                                     ident[:], is_transpose=True)
                    nc.vector.tensor_copy(xT[:, k * 128:(k + 1) * 128], ps[:])
                psv = ps_sh.tile([128, D], F32, tag="s")
                for k in range(2):
                    nc.tensor.matmul(psv[:], xT[:, k * 128:(k + 1) * 128],
                                     WvT[k][:], start=(k == 0), stop=(k == 1))
                nc.vector.tensor_tensor(vbat[:, yb * D:(yb + 1) * D], psv[:],
                                        bval_full[:], AL.add)
            # write tables: per (h, cp) one DMA covering 8 rows
            y0 = batch * 8
            for h in range(NH):
                for cp in range(2):
                    off = cp * (NE * 64) + (T0(y0) - cp) * 32
                    # dst dims (x:128, yb:8, c:32); src (x part, yb, c)
                    dst = tabf[h][off:off + 8 * WP * 32]
                    dst = dst.rearrange("(yb r) -> yb r", r=WP * 32)  # 8 x row
                    # take first 128*32 of each row-block: [yb, x, c]
                    dst = dst[:, 0:128 * 32].rearrange("yb (x c) -> x yb c", c=32)
                    nc.sync.dma_start(
                        dst,
                        vbat[:].rearrange("x (yb hh c) -> x yb hh c", yb=8, c=32)
                        [:, :, h, :])

        # ---------------- phase 2: attention + FFN ----------------
        for g in range(NG):
            # -- per-tile projections and sampling arithmetic --
            xr_t, wcomb_t, oT_t = [], [], []
            idxQg = p2.tile([128, 256], F32, tag="idxQg")
            for T in range(4):
                r_loc = g * 4 + T
                xr = p2.tile([128, D], F32R, tag=f"p2_x{T}")
                nc.sync.dma_start(xr[:], src_q[r_loc * W:(r_loc + 1) * W, :])
                xr_t.append(xr)
                xT = p2.tile([128, D], F32R, tag="p2_xT")
                for k in range(2):
                    ps = ps_sh.tile([128, 128], F32R, tag="s")
                    nc.tensor.matmul(ps[:], xr[:, k * 128:(k + 1) * 128],
                                     ident[:], is_transpose=True)
                    nc.vector.tensor_copy(xT[:, k * 128:(k + 1) * 128], ps[:])
                # off/attn projection [128 q, 96]
                psoa = ps_sh.tile([128, 96], F32, tag="s")
                for k in range(2):
                    nc.tensor.matmul(psoa[:], xT[:, k * 128:(k + 1) * 128],
                                     WoaT[k][:], start=(k == 0), stop=(k == 1))
                oa = p2.tile([128, 96], F32, tag="p2_oab")
                nc.vector.tensor_tensor(oa[:], psoa[:], boa_full[:], AL.add)

                def tl(name, shape=(128, 32)):
                    return p2.tile(list(shape), F32, tag=name,
                                   name=f"{name}_{g}_{T}")

                # px = off_x + q; py = off_y + rowbase + r_loc
                px = tl("px"); py = tl("py")
                nc.vector.tensor_scalar_add(px[:], oa[:, 0:64:2], ix[:])
                nc.vector.tensor_scalar(py[:], oa[:, 1:64:2], rb[:],
                                        float(r_loc), AL.add, AL.add)

                def floor_of(src_ap, shift, nm):
                    w = tl("fl_w_" + nm)
                    nc.vector.tensor_scalar_add(w[:], src_ap, float(shift))
                    wi = p2.tile([128, 32], I32, tag="fl_i_" + nm, name=f"fl_i_{nm}_{g}_{T}")
                    nc.vector.tensor_copy(wi[:], w[:])
                    wf = tl("fl_f_" + nm)
                    nc.vector.tensor_copy(wf[:], wi[:])
                    cg = tl("fl_c_" + nm)
                    nc.vector.tensor_tensor(cg[:], wf[:], w[:], AL.is_gt)
                    nc.vector.tensor_tensor(cg[:], wf[:], cg[:], AL.subtract)
                    if shift:
                        nc.vector.tensor_scalar_add(cg[:], cg[:], -float(shift))
                    return cg

                x0 = floor_of(px[:], 16.0, "x")
                y0 = floor_of(py[:], 16.0, "y")
                wx = tl("wx"); wy = tl("wy")
                nc.vector.tensor_tensor(wx[:], px[:], x0[:], AL.subtract)
                nc.vector.tensor_tensor(wy[:], py[:], y0[:], AL.subtract)
                x0c = tl("x0c"); y0c = tl("y0c")
                nc.vector.tensor_scalar(x0c[:], x0[:], -2.0, 128.0, AL.max, AL.min)
                nc.vector.tensor_scalar(y0c[:], y0[:], -2.0, 128.0, AL.max, AL.min)

                # attn softmax over p (groups of 4 cols)
                att = oa[:, 64:96]
                mx = p2.tile([128, 8], F32, tag="sm_mx")
                nc.vector.tensor_reduce(
                    mx[:], att.rearrange("q (h p) -> q h p", p=4), AX.X, AL.max)
                e = tl("sm_e")
                nc.vector.tensor_tensor(
                    e[:].rearrange("q (h p) -> q h p", p=4),
                    att.rearrange("q (h p) -> q h p", p=4),
                    mx[:].unsqueeze(-1).broadcast_to((128, 8, 4)), AL.subtract)
                nc.scalar.activation(e[:], e[:], AF.Exp)
                sm = p2.tile([128, 8], F32, tag="sm_s")
                nc.vector.tensor_reduce(
                    sm[:], e[:].rearrange("q (h p) -> q h p", p=4), AX.X, AL.add)
                nc.vector.reciprocal(sm[:], sm[:])
                aw = tl("sm_aw")
                nc.vector.tensor_tensor(
                    aw[:].rearrange("q (h p) -> q h p", p=4),
                    e[:].rearrange("q (h p) -> q h p", p=4),
                    sm[:].unsqueeze(-1).broadcast_to((128, 8, 4)), AL.mult)

                # corner weights -> wcomb [128, 128] col = h*16 + p*4 + side*2 + lr
                wxl = tl("wxl")
                nc.vector.tensor_scalar(wxl[:], wx[:], -1.0, 1.0, AL.mult, AL.add)
                awt = tl("awt"); awb = tl("awb")
                nc.vector.tensor_tensor(awb[:], aw[:], wy[:], AL.mult)
                nc.vector.tensor_tensor(awt[:], aw[:], awb[:], AL.subtract)
                wcomb = p2.tile([128, 128], F32, tag=f"wcomb{T}")
                wc4 = wcomb[:].rearrange("q (hp s l) -> q hp s l", s=2, l=2)
                nc.vector.tensor_tensor(wc4[:, :, 0, 0], awt[:], wxl[:], AL.mult)
                nc.vector.tensor_tensor(wc4[:, :, 0, 1], awt[:], wx[:], AL.mult)
                nc.vector.tensor_tensor(wc4[:, :, 1, 0], awb[:], wxl[:], AL.mult)
                nc.vector.tensor_tensor(wc4[:, :, 1, 1], awb[:], wx[:], AL.mult)
                wcomb_t.append(wcomb)

                # indices: t0 = (y0c+2)*WP + x0c + 2
                t0 = tl("t0")
                nc.vector.tensor_scalar(t0[:], y0c[:], float(WP), float(2 * WP + 2),
                                        AL.mult, AL.add)
                nc.vector.tensor_tensor(t0[:], t0[:], x0c[:], AL.add)
                th = tl("th")
                nc.vector.tensor_scalar_mul(th[:], t0[:], 0.5)
                half = floor_of(th[:], 0.0, "h")
                odd = tl("odd")
                nc.vector.tensor_scalar(odd[:], half[:], -2.0, 0.0, AL.mult, AL.add)
                nc.vector.tensor_tensor(odd[:], t0[:], odd[:], AL.add)
                itop = tl("itop")
                nc.vector.tensor_scalar_mul(itop[:], odd[:], float(NE))
                nc.vector.tensor_tensor(itop[:], itop[:], half[:], AL.add)
                # write into idxQg cols T*64 + (hp)*2 + side
                iq4 = idxQg[:, T * 64:(T + 1) * 64].rearrange(
                    "q (hp s) -> q hp s", s=2)
                nc.vector.tensor_copy(iq4[:, :, 0], itop[:])
                nc.vector.tensor_scalar_add(iq4[:, :, 1], itop[:], float(WP // 2))

            # -- selector matmuls -> idxbig [128, 2048] i16 --
            idxbig = p2.tile([128, NH * 256], I16, tag="idxbig", bufs=1)
            for m in range(8):
                psi = ps_sh.tile([128, 256], F32, tag="s")
                nc.tensor.matmul(psi[:], sels[m][:], idxQg[:])
                # psi[p', T*64 + h*8 + kk] -> idxbig[:, h*256 + kk*32 + T*8 + m]
                nc.vector.tensor_copy(
                    idxbig[:].rearrange("q (h kk s) -> q h kk s", h=NH, kk=8)
                    [:, :, :, m:32:8],
                    psi[:].rearrange("q (t h kk) -> q h kk t", t=4, h=NH))

            # -- per-head gather + combine --
            o_t = [p2.tile([128, D], F32, tag=f"oT{T}", name=f"o_t{g}_{T}") for T in range(4)]
            for h in range(NH):
                gh = gp.tile([128, 32, 64], F32, tag="G", bufs=2)
                if SKIP_GATHER:
                    nc.gpsimd.memset(gh[:], 0.125)
                else:
                    nc.gpsimd.dma_gather(
                        gh[:], tabs[h][:],
                        idxbig[:, h * 256:(h + 1) * 256],
                        4096, 4096, 64, single_packet=False)
                for T in range(4):
                    pt = gp.tile([128, 512], F32, tag="P")
                    nc.vector.tensor_tensor(
                        pt[:].rearrange("q (c kk l) -> q kk l c", c=32, kk=8),
                        gh[:, T:32:4, :].rearrange("q kk (l c) -> q kk l c", l=2),
                        wcomb_t[T][:, h * 16:(h + 1) * 16]
                        .rearrange("q (kk l) -> q kk l", l=2)
                        .unsqueeze(-1).broadcast_to((128, 8, 2, 32)),
                        AL.mult)
                    nc.vector.tensor_reduce(
                        o_t[T][:, h * 32:(h + 1) * 32],
                        pt[:].rearrange("q (c s) -> q c s", s=16), AX.X, AL.add)

            # -- out-proj, LN1, FFN, LN2 per tile --
            h1T_g = [p2.tile([128, 512], F32R, tag=f"h1T{ck}", name=f"h1T{g}_{ck}", bufs=1) for ck in range(2)]
            h1_t = []
            for T in range(4):
                oT = p2.tile([128, D], F32R, tag="oTT")
                for k in range(2):
                    ps = ps_sh.tile([128, 128], F32, tag="s")
                    nc.tensor.matmul(ps[:], o_t[T][:, k * 128:(k + 1) * 128],
                                     ident_f[:], is_transpose=True)
                    nc.vector.tensor_copy(oT[:, k * 128:(k + 1) * 128], ps[:])
                ps2 = ps_sh.tile([128, D], F32, tag="s")
                for k in range(2):
                    nc.tensor.matmul(ps2[:], oT[:, k * 128:(k + 1) * 128],
                                     WoT[k][:], start=(k == 0), stop=(k == 1))

                def layer_norm(ps_in, resid_ap, bias_full, gfull, befull, outdt, nm):
                    t2 = p2.tile([128, D], F32, tag="ln_t2", name=f"lnt2_{nm}_{g}")
                    nc.vector.tensor_tensor(t2[:], ps_in, bias_full[:], AL.add)
                    nc.vector.tensor_tensor(t2[:], t2[:], resid_ap, AL.add)
                    mu = p2.tile([128, 1], F32, tag="ln_mu")
                    nc.vector.tensor_reduce(mu[:], t2[:], AX.X, AL.add)
                    nc.vector.tensor_scalar_mul(mu[:], mu[:], 1.0 / D)
                    nmu = p2.tile([128, 1], F32, tag="ln_nmu")
                    nc.vector.tensor_scalar_mul(nmu[:], mu[:], -1.0)
                    sq = p2.tile([128, D], F32, tag="ln_sq", name=f"lnsq_{nm}_{g}")
                    var = p2.tile([128, 1], F32, tag="ln_var")
                    nc.scalar.activation(sq[:], t2[:], AF.Square,
                                         bias=nmu[:], scale=1.0, accum_out=var[:])
                    rs = p2.tile([128, 1], F32, tag="ln_rs")
                    nc.vector.tensor_scalar(rs[:], var[:], 1.0 / D, 1e-5,
                                            AL.mult, AL.add)
                    nc.scalar.activation(rs[:], rs[:], AF.Sqrt)
                    nc.vector.reciprocal(rs[:], rs[:])
                    nb = p2.tile([128, 1], F32, tag="ln_nb")
                    nc.vector.tensor_tensor(nb[:], nmu[:], rs[:], AL.mult)
                    z = p2.tile([128, D], F32, tag="ln_z", name=f"lnz_{nm}_{g}")
                    nc.vector.tensor_scalar(z[:], t2[:], rs[:], nb[:],
                                            AL.mult, AL.add)
                    o = p2.tile([128, D], outdt, tag=f"ln_o_{nm}")
                    nc.vector.tensor_tensor(z[:], z[:], gfull[:], AL.mult)
                    nc.vector.tensor_tensor(o[:], z[:], befull[:], AL.add)
                    return o

                h1 = layer_norm(ps2[:], xr_t[T][:].bitcast(F32), bout_full,
                                g1_full, be1_full, F32, f"1_{T}")
                h1_t.append(h1)
                for k in range(2):
                    ps = ps_sh.tile([128, 128], F32, tag="s")
                    nc.tensor.matmul(ps[:], h1[:, k * 128:(k + 1) * 128],
                                     ident_f[:], is_transpose=True)
                    nc.vector.tensor_copy(h1T_g[k][:, T * 128:(T + 1) * 128], ps[:])

            # FFN over the group
            rlu = [gp.tile([128, 512], F32R, tag=f"rlu{dfc}", name=f"rlu{g}_{dfc}", bufs=1) for dfc in range(16)]
            for dfc in range(16):
                psf = ps_f.tile([128, 512], F32, tag="psf")
                for ck in range(2):
                    nc.tensor.matmul(psf[:], W1T[ck][dfc][:], h1T_g[ck][:],
                                     start=(ck == 0), stop=(ck == 1))
                nc.scalar.activation(rlu[dfc][:], psf[:], AF.Relu,
                                     bias=b1t[:, dfc:dfc + 1], scale=1.0)
            for T in range(4):
                pso = ps_o.tile([128, D], F32, tag="pso")
                for dfc in range(16):
                    nc.tensor.matmul(pso[:], rlu[dfc][:, T * 128:(T + 1) * 128],
                                     W2T[dfc][:], start=(dfc == 0), stop=(dfc == 15))
                o2 = layer_norm(pso[:], h1_t[T][:].bitcast(F32), b2_full,
                                g2_full, be2_full, F32, "2")
                r_loc = g * 4 + T
                nc.sync.dma_start(out[r_loc * W:(r_loc + 1) * W, :], o2[:])

        for cm in reversed(phase_cm):
            cm.__exit__(None, None, None)

    nc.compile()
    return nc


def kernel(**inputs):
    global _nc_cache
    if _nc_cache is None:
        _nc_cache = build_nc()
    nc = _nc_cache

    src = np.asarray(inputs["src"], np.float32)          # [2,128,128,256]
    f = lambda k: np.ascontiguousarray(np.asarray(inputs[k], np.float32))
    row = lambda k: f(k).reshape(1, -1)

    in_maps = []
    for c in range(8):
        b, j = c // 4, c % 4
        m = {
            "src_img": np.ascontiguousarray(src[b].reshape(H * W, D)),
            "src_q": np.ascontiguousarray(
                src[b, 32 * j:32 * (j + 1)].reshape(QROWS * W, D)),
            "rowbase": np.array([[32.0 * j]], np.float32),
            "w_off": f("W_off"), "b_off": row("b_off"),
            "w_attn": f("W_attn"), "b_attn": row("b_attn"),
            "w_val": f("W_val"), "b_val": row("b_val"),
            "w_out": f("W_out"), "b_out": row("b_out"),
            "w1": f("W1"), "b1": row("b1"),
            "w2": f("W2"), "b2": row("b2"),
            "g1": row("g1"), "be1": row("be1"),
            "g2": row("g2"), "be2": row("be2"),
        }
        in_maps.append(m)

    trace = bool(int(os.environ.get("K_TRACE", "0")))
    res = run_bass_kernel_spmd(nc, in_maps, core_ids=list(range(8)),
                               trace=trace)
    if trace:
        kernel.last_exec_time_ns = res.exec_time_ns
        kernel.last_trace = res.instructions_and_trace
    else:
        kernel.last_exec_time_ns = None
    quarters = [r["out"].reshape(QROWS, W, D) for r in
                (res.results[c] for c in range(8))]
    img0 = np.concatenate(quarters[0:4], axis=0)
    img1 = np.concatenate(quarters[4:8], axis=0)
    return np.stack([img0, img1], axis=0)


# revision 22
# speedup vs baseline: 38.7290x; 38.7290x over previous
"""Deformable transformer encoder layer on 8 Trainium2 NeuronCores.

Sharding: core c handles batch b=c//4, query rows [32j, 32j+32), j=c%4.
Each core builds full-image per-head bilinear gather tables (dual-copy,
256B elements), samples with SWDGE dma_gather, then out-proj + LN + FFN
+ LN for its 4096 queries. Host only splits inputs / concats outputs.
"""
import sys, os
sys.path.insert(0, '/opt/trn_rl_repo')
SKIP_GATHER = bool(int(os.environ.get("K_SKIP_GATHER", "0")))
IDX0 = bool(int(os.environ.get("K_IDX0", "0")))
import numpy as np

import concourse.bass as bass
import concourse.bacc as bacc
import concourse.mybir as mybir
import concourse.tile as tile
from concourse import masks
from concourse.bass_utils import run_bass_kernel_spmd

F32 = mybir.dt.float32
F32R = mybir.dt.float32r
I32 = mybir.dt.int32
I16 = mybir.dt.int16
AL = mybir.AluOpType
AF = mybir.ActivationFunctionType
AX = mybir.AxisListType

D, NH, DH, NP, DFF = 256, 8, 32, 4, 2048
B, H, W = 2, 128, 128
HP, WP = H + 4, W + 4            # zero-padded image dims
NPAD = HP * WP                   # 17424 padded tokens
NE = NPAD // 2                   # 8712 elements per table copy
QROWS = 32                       # query rows per core
NG = 8                           # groups of 4 query tiles

_nc_cache = None


def build_nc():
    nc = bacc.Bacc(None, target_bir_lowering=False)

    src_img = nc.dram_tensor("src_img", [H * W, D], F32R, kind="ExternalInput")
    src_q = nc.dram_tensor("src_q", [QROWS * W, D], F32R, kind="ExternalInput")
    rowbase = nc.dram_tensor("rowbase", [1, 1], F32, kind="ExternalInput")
    w_off = nc.dram_tensor("w_off", [NH * NP * 2, D], F32R, kind="ExternalInput")
    b_off = nc.dram_tensor("b_off", [1, NH * NP * 2], F32, kind="ExternalInput")
    w_attn = nc.dram_tensor("w_attn", [NH * NP, D], F32R, kind="ExternalInput")
    b_attn = nc.dram_tensor("b_attn", [1, NH * NP], F32, kind="ExternalInput")
    w_val = nc.dram_tensor("w_val", [D, D], F32R, kind="ExternalInput")
    b_val = nc.dram_tensor("b_val", [1, D], F32, kind="ExternalInput")
    w_out = nc.dram_tensor("w_out", [D, D], F32R, kind="ExternalInput")
    b_out = nc.dram_tensor("b_out", [1, D], F32, kind="ExternalInput")
    w1 = nc.dram_tensor("w1", [DFF, D], F32R, kind="ExternalInput")
    b1 = nc.dram_tensor("b1", [1, DFF], F32, kind="ExternalInput")
    w2 = nc.dram_tensor("w2", [D, DFF], F32R, kind="ExternalInput")
    b2 = nc.dram_tensor("b2", [1, D], F32, kind="ExternalInput")
    g1 = nc.dram_tensor("g1", [1, D], F32, kind="ExternalInput")
    be1 = nc.dram_tensor("be1", [1, D], F32, kind="ExternalInput")
    g2 = nc.dram_tensor("g2", [1, D], F32, kind="ExternalInput")
    be2 = nc.dram_tensor("be2", [1, D], F32, kind="ExternalInput")
    out = nc.dram_tensor("out", [QROWS * W, D], F32, kind="ExternalOutput")
    # per-head dual-copy gather tables, elements of 64 f32 (256B)
    tabs = [nc.dram_tensor(f"tabs{h}", [2 * NE, 64], F32) for h in range(NH)]
    tabf = [t[:].rearrange("e c -> (e c)") for t in tabs]  # flat f32 units

    def T0(y):  # first padded token of image row y
        return (y + 2) * WP + 2

    with tile.TileContext(nc) as tc:
      with (
        tc.tile_pool(name="const", bufs=1) as cpool,
        tc.tile_pool(name="wpool", bufs=1) as wpool,
        tc.tile_pool(name="ps_sh", bufs=3, space="PSUM") as ps_sh,
      ):
        # ---------------- constants ----------------
        ident_f = cpool.tile([128, 128], F32)
        masks.make_identity(nc, ident_f[:])
        ident = cpool.tile([128, 128], F32R)
        nc.vector.tensor_copy(ident[:], ident_f[:])

        sels = []
        for m in range(8):
            s = cpool.tile([128, 128], F32, tag=f"sel{m}")
            nc.gpsimd.memset(s[:], 0.0)
            nc.gpsimd.affine_select(
                out=s[:], in_=s[:], compare_op=AL.not_equal, fill=1.0,
                base=-16 * m, pattern=[[0, 8], [-1, 16]], channel_multiplier=1)
            sels.append(s)

        ones_r = cpool.tile([1, 128], F32)
        nc.vector.memset(ones_r[:], 1.0)

        iox = cpool.tile([128, 1], I32)
        nc.gpsimd.iota(iox[:], pattern=[[1, 1]], base=0, channel_multiplier=1)
        ix = cpool.tile([128, 1], F32)
        nc.vector.tensor_copy(ix[:], iox[:])

        def bcast_cols(dram_row, n, name):
            """[1, n] DRAM -> [128, n] SBUF (outer product with ones)."""
            r = cpool.tile([1, n], F32, tag=f"ob_{name}")
            nc.sync.dma_start(r[:], dram_row)
            ps = ps_sh.tile([128, n], F32, tag="s")
            nc.tensor.matmul(ps[:], ones_r[:], r[:])
            t = cpool.tile([128, n], F32, tag=f"obf_{name}")
            nc.vector.tensor_copy(t[:], ps[:])
            return t

        rb = bcast_cols(rowbase[:], 1, "rb")          # [128,1] = 32*j
        boa_full = cpool.tile([128, 96], F32)
        nc.vector.tensor_copy(boa_full[:, 0:64], bcast_cols(b_off[:], 64, "boff")[:])
        nc.vector.tensor_copy(boa_full[:, 64:96], bcast_cols(b_attn[:], 32, "batt")[:])
        bval_full = bcast_cols(b_val[:], D, "bval")
        bout_full = bcast_cols(b_out[:], D, "bout")
        b2_full = bcast_cols(b2[:], D, "b2")
        g1_full = bcast_cols(g1[:], D, "g1")
        be1_full = bcast_cols(be1[:], D, "be1")
        g2_full = bcast_cols(g2[:], D, "g2")
        be2_full = bcast_cols(be2[:], D, "be2")

        # b1 in ff1T layout: b1t[p, dfc] = b1[dfc*128 + p]
        b1t = cpool.tile([128, 16], F32)
        nc.sync.dma_start(
            b1t[:], b1[:].rearrange("a (hi p) -> a hi p", p=128)[0].transpose([1, 0]))

        # ---------------- weight transposes ----------------
        wload_cm = tc.tile_pool(name="wload", bufs=1)
        wload = wload_cm.__enter__()

        def load_transpose(wdram, rows, name):
            """[rows=256, 256] DRAM -> 2 f32r tiles [128 (d chunk), 256] = W.T"""
            assert rows == 256
            outs = [wpool.tile([128, rows], F32R, tag=f"ltT_{name}{k}", name=f"ltT_{name}{k}")
                    for k in range(2)]
            for rc in range(2):
                wsb = wload.tile([128, D], F32R, tag=f"lt_{name}{rc}")
                nc.sync.dma_start(wsb[:], wdram[rc * 128:(rc + 1) * 128, :])
                for k in range(2):
                    ps = ps_sh.tile([128, 128], F32R, tag="s")
                    nc.tensor.matmul(ps[:], wsb[:, k * 128:(k + 1) * 128],
                                     ident[:], is_transpose=True)
                    nc.vector.tensor_copy(
                        outs[k][:, rc * 128:(rc + 1) * 128], ps[:])
            return outs

        WvT = load_transpose(w_val[:], D, "wv")
        WoT = load_transpose(w_out[:], D, "wo")
        woa = wload.tile([96, D], F32R)
        nc.sync.dma_start(woa[0:64, :], w_off[:])
        nc.sync.dma_start(woa[64:96, :], w_attn[:])
        WoaT = []
        for k in range(2):
            ps = ps_sh.tile([128, 96], F32R, tag="s")
            nc.tensor.matmul(ps[:, 0:96], woa[:, k * 128:(k + 1) * 128],
                             ident[0:96, 0:96], is_transpose=True)
            t = wpool.tile([128, 96], F32R, tag=f"woaT{k}")
            nc.vector.tensor_copy(t[:], ps[:, 0:96])
            WoaT.append(t)

        W1T = [[None] * 16 for _ in range(2)]   # [ck][dfc]: [128 c, 128 dff]
        for dfc in range(16):
            wsb = wload.tile([128, D], F32R, tag="w1_ld")
            nc.sync.dma_start(wsb[:], w1[dfc * 128:(dfc + 1) * 128, :])
            for ck in range(2):
                ps = ps_sh.tile([128, 128], F32R, tag="s")
                nc.tensor.matmul(ps[:], wsb[:, ck * 128:(ck + 1) * 128],
                                 ident[:], is_transpose=True)
                t = wpool.tile([128, 128], F32R, tag=f"w1T_{ck}_{dfc}")
                nc.vector.tensor_copy(t[:], ps[:])
                W1T[ck][dfc] = t

        W2T = []                                 # [dfc]: [128 dff, 256 c]
        w2sb = []
        for ck in range(2):
            wsb = wload.tile([128, DFF], F32R, tag=f"w2_ld{ck}")
            nc.sync.dma_start(wsb[:], w2[ck * 128:(ck + 1) * 128, :])
            w2sb.append(wsb)
        for dfc in range(16):
            t = wpool.tile([128, D], F32R, tag=f"w2T_{dfc}")
            for ck in range(2):
                ps = ps_sh.tile([128, 128], F32R, tag="s")
                nc.tensor.matmul(ps[:], w2sb[ck][:, dfc * 128:(dfc + 1) * 128],
                                 ident[:], is_transpose=True)
                nc.vector.tensor_copy(t[:, ck * 128:(ck + 1) * 128], ps[:])
            W2T.append(t)

        wload_cm.__exit__(None, None, None)
        phase_cm = []
        def _open(name, **kw):
            cm = tc.tile_pool(name=name, **kw)
            phase_cm.append(cm)
            return cm.__enter__()
        p1 = _open("p1", bufs=3)
        p2 = _open("p2", bufs=2)
        gp = _open("gp", bufs=3)
        ps_f = _open("ps_f", bufs=2, space="PSUM")
        ps_o = _open("ps_o", bufs=2, space="PSUM")

        # ---------------- zero-fill table pads ----------------
        zt = cpool.tile([128, 128], F32)
        nc.vector.memset(zt[:], 0.0)

        def zero_fill(h, cp, tok0, ntok):
            """write ntok*32 zero f32 at token tok0 of table copy cp."""
            off = cp * (NE * 64) + (tok0 - cp) * 32
            left = ntok
            while left > 0:
                a = min(128, left)
                nc.sync.dma_start(
                    tabf[h][off:off + a * 32].rearrange("(a b) -> a b", b=32),
                    zt[0:a, 0:32])
                off += a * 32
                left -= a

        for h in range(NH):
            for cp in range(2):
                zero_fill(h, cp, cp, T0(0) - cp)                 # leading
                zero_fill(h, cp, T0(127) + W, NPAD + cp - (T0(127) + W))  # trailing
                # row gaps: 127 runs of 4 tokens, stride 132 tokens
                gap0 = cp * (NE * 64) + (T0(0) + W - cp) * 32
                nc.sync.dma_start(
                    tabf[h][gap0:gap0 + 127 * WP * 32]
                    .rearrange("(y r) -> y r", r=WP * 32)[:, 0:128],
                    zt[0:127, 0:128])

        # ---------------- phase 1: value + tables ----------------
        for batch in range(16):
            vbat = p1.tile([128, 8 * D], F32, tag="vbat", bufs=2)
            for yb in range(8):
                y = batch * 8 + yb
                xr = p1.tile([128, D], F32R, tag="p1_x")
                nc.sync.dma_start(xr[:], src_img[y * W:(y + 1) * W, :])
                xT = p1.tile([128, D], F32R, tag="p1_xT")
                for k in range(2):
                    ps = ps_sh.tile([128, 128], F32R, tag="s")
                    nc.tensor.matmul(ps[:], xr[:, k * 128:(k + 1) * 128],
                                     ident[:], is_transpose=True)
                    nc.vector.tensor_copy(xT[:, k * 128:(k + 1) * 128], ps[:])
                psv = ps_sh.tile([128, D], F32, tag="s")
                for k in range(2):
                    nc.tensor.matmul(psv[:], xT[:, k * 128:(k + 1) * 128],
                                     WvT[k][:], start=(k == 0), stop=(k == 1))
                nc.vector.tensor_tensor(vbat[:, yb * D:(yb + 1) * D], psv[:],
                                        bval_full[:], AL.add)
            # write tables: per (h, cp) one DMA covering 8 rows
            y0 = batch * 8
            for h in range(NH):
                for cp in range(2):
                    off = cp * (NE * 64) + (T0(y0) - cp) * 32
                    # dst dims (x:128, yb:8, c:32); src (x part, yb, c)
                    dst = tabf[h][off:off + 8 * WP * 32]
                    dst = dst.rearrange("(yb r) -> yb r", r=WP * 32)  # 8 x row
                    # take first 128*32 of each row-block: [yb, x, c]
                    dst = dst[:, 0:128 * 32].rearrange("yb (x c) -> x yb c", c=32)
                    nc.sync.dma_start(
                        dst,
                        vbat[:].rearrange("x (yb hh c) -> x yb hh c", yb=8, c=32)
                        [:, :, h, :])

        # ---------------- phase 2: attention + FFN ----------------
        # tileT const: [128, 4] with value T in column T
        tofi = cpool.tile([128, 4], I32)
        nc.gpsimd.iota(tofi[:], pattern=[[1, 4]], base=0, channel_multiplier=0)
        tof4 = cpool.tile([128, 4], F32)
        nc.vector.tensor_copy(tof4[:], tofi[:])

        for g in range(NG):
            xr_t = []
            oaG = p2.tile([128, 4, 96], F32, tag="oaG", bufs=1)
            for T in range(4):
                r_loc = g * 4 + T
                xr = p2.tile([128, D], F32R, tag=f"p2_x{T}")
                nc.sync.dma_start(xr[:], src_q[r_loc * W:(r_loc + 1) * W, :])
                xr_t.append(xr)
                xT = p2.tile([128, D], F32R, tag="p2_xT")
                for k in range(2):
                    ps = ps_sh.tile([128, 128], F32R, tag="s")
                    nc.tensor.matmul(ps[:], xr[:, k * 128:(k + 1) * 128],
                                     ident[:], is_transpose=True)
                    nc.vector.tensor_copy(xT[:, k * 128:(k + 1) * 128], ps[:])
                psoa = ps_sh.tile([128, 96], F32, tag="s")
                for k in range(2):
                    nc.tensor.matmul(psoa[:], xT[:, k * 128:(k + 1) * 128],
                                     WoaT[k][:], start=(k == 0), stop=(k == 1))
                nc.vector.tensor_tensor(oaG[:, T, :], psoa[:], boa_full[:], AL.add)

            def gt(name, shape=(128, 4, 32)):
                return p2.tile(list(shape), F32, tag=name, name=f"{name}_{g}",
                               bufs=1)

            # px = off_x + q ; py = off_y + rowbase + 4g + T
            px = gt("px"); py = gt("py")
            nc.vector.tensor_scalar_add(px[:], oaG[:, :, 0:64:2], ix[:])
            nc.vector.tensor_scalar(py[:], oaG[:, :, 1:64:2], rb[:],
                                    float(4 * g), AL.add, AL.add)
            nc.vector.tensor_tensor(
                py[:], py[:], tof4[:].unsqueeze(-1).broadcast_to((128, 4, 32)),
                AL.add)

            def floor_of(src_ap, shift, nm):
                w = gt("fl_w_" + nm)
                nc.vector.tensor_scalar_add(w[:], src_ap, float(shift))
                wi = p2.tile([128, 4, 32], I32, tag="fl_i_" + nm,
                             name=f"fl_i_{nm}_{g}")
                nc.vector.tensor_copy(wi[:], w[:])
                wf = gt("fl_f_" + nm)
                nc.vector.tensor_copy(wf[:], wi[:])
                cg = gt("fl_c_" + nm)
                nc.vector.tensor_tensor(cg[:], wf[:], w[:], AL.is_gt)
                nc.vector.tensor_tensor(cg[:], wf[:], cg[:], AL.subtract)
                if shift:
                    nc.vector.tensor_scalar_add(cg[:], cg[:], -float(shift))
                return cg

            x0 = floor_of(px[:], 16.0, "x")
            y0 = floor_of(py[:], 16.0, "y")
            wx = gt("wx"); wy = gt("wy")
            nc.vector.tensor_tensor(wx[:], px[:], x0[:], AL.subtract)
            nc.vector.tensor_tensor(wy[:], py[:], y0[:], AL.subtract)
            x0c = gt("x0c"); y0c = gt("y0c")
            nc.vector.tensor_scalar(x0c[:], x0[:], -2.0, 128.0, AL.max, AL.min)
            nc.vector.tensor_scalar(y0c[:], y0[:], -2.0, 128.0, AL.max, AL.min)

            # softmax over p within each (T, h)
            att = oaG[:, :, 64:96]            # [128, 4, 32]
            mx = p2.tile([128, 4, 8], F32, tag="sm_mx", name=f"sm_mx_{g}")
            nc.vector.tensor_reduce(
                mx[:], att.rearrange("q t (h p) -> q t h p", p=4), AX.X, AL.max)
            e = gt("sm_e")
            nc.vector.tensor_tensor(
                e[:].rearrange("q t (h p) -> q t h p", p=4),
                att.rearrange("q t (h p) -> q t h p", p=4),
                mx[:].unsqueeze(-1).broadcast_to((128, 4, 8, 4)), AL.subtract)
            nc.scalar.activation(e[:], e[:], AF.Exp)
            sm = p2.tile([128, 4, 8], F32, tag="sm_s", name=f"sm_s_{g}")
            nc.vector.tensor_reduce(
                sm[:], e[:].rearrange("q t (h p) -> q t h p", p=4), AX.X, AL.add)
            nc.vector.reciprocal(sm[:], sm[:])
            aw = gt("sm_aw")
            nc.vector.tensor_tensor(
                aw[:].rearrange("q t (h p) -> q t h p", p=4),
                e[:].rearrange("q t (h p) -> q t h p", p=4),
                sm[:].unsqueeze(-1).broadcast_to((128, 4, 8, 4)), AL.mult)

            # corner weights into wcombG[q, T*128 + h*16 + p*4 + side*2 + lr]
            wxl = gt("wxl")
            nc.vector.tensor_scalar(wxl[:], wx[:], -1.0, 1.0, AL.mult, AL.add)
            awt = gt("awt"); awb = gt("awb")
            nc.vector.tensor_tensor(awb[:], aw[:], wy[:], AL.mult)
            nc.vector.tensor_tensor(awt[:], aw[:], awb[:], AL.subtract)
            wcombG = p2.tile([128, 512], F32, tag="wcombG")
            wc4 = wcombG[:].rearrange("q (t hp s l) -> q t hp s l", t=4, s=2, l=2)
            nc.vector.tensor_tensor(wc4[:, :, :, 0, 0], awt[:], wxl[:], AL.mult)
            nc.vector.tensor_tensor(wc4[:, :, :, 0, 1], awt[:], wx[:], AL.mult)
            nc.vector.tensor_tensor(wc4[:, :, :, 1, 0], awb[:], wxl[:], AL.mult)
            nc.vector.tensor_tensor(wc4[:, :, :, 1, 1], awb[:], wx[:], AL.mult)

            # indices
            t0v = gt("t0v")
            nc.vector.tensor_scalar(t0v[:], y0c[:], float(WP), float(2 * WP + 2),
                                    AL.mult, AL.add)
            nc.vector.tensor_tensor(t0v[:], t0v[:], x0c[:], AL.add)
            th = gt("th")
            nc.vector.tensor_scalar_mul(th[:], t0v[:], 0.5)
            half = floor_of(th[:], 0.0, "h")
            odd = gt("odd")
            nc.vector.tensor_scalar(odd[:], half[:], -2.0, 0.0, AL.mult, AL.add)
            nc.vector.tensor_tensor(odd[:], t0v[:], odd[:], AL.add)
            itop = gt("itop")
            nc.vector.tensor_scalar_mul(itop[:], odd[:], float(NE))
            nc.vector.tensor_tensor(itop[:], itop[:], half[:], AL.add)
            # idxQg[q, T*64 + h*8 + p*2 + side]
            idxQg = p2.tile([128, 256], F32, tag="idxQg")
            iq4 = idxQg[:].rearrange("q (t hp s) -> q t hp s", t=4, s=2)
            nc.vector.tensor_copy(iq4[:, :, :, 0], itop[:])
            nc.vector.tensor_scalar_add(iq4[:, :, :, 1], itop[:], float(WP // 2))

            # selector matmuls -> idxbig
            idxbig = p2.tile([128, NH * 256], I16, tag="idxbig", bufs=1)
            for m in range(8):
                psi = ps_sh.tile([128, 256], F32, tag="s")
                nc.tensor.matmul(psi[:], sels[m][:], idxQg[:])
                nc.vector.tensor_copy(
                    idxbig[:].rearrange("q (h kk s) -> q h kk s", h=NH, kk=8)
                    [:, :, :, m:32:8],
                    psi[:].rearrange("q (t h kk) -> q h kk t", t=4, h=NH))

            # per-head gather + combine into OG[q, T*256 + h*32 + c]
            OG = p2.tile([128, 1024], F32, tag="OG", bufs=1)
            for h in range(NH):
                gh = gp.tile([128, 32, 64], F32, tag="G", bufs=2)
                if SKIP_GATHER:
                    nc.gpsimd.memset(gh[:], 0.125)
                else:
                    nc.gpsimd.dma_gather(
                        gh[:], tabs[h][:],
                        idxbig[:, h * 256:(h + 1) * 256],
                        4096, 4096, 64, single_packet=False)
                pt = gp.tile([128, 2048], F32, tag="P", name=f"P_{g}_{h}", bufs=1)
                ptv = pt[:].rearrange("q (t c kk l) -> q kk t l c",
                                      t=4, c=32, kk=8)
                ghv = gh[:].rearrange("q (kk t) (l c) -> q kk t l c", t=4, l=2)
                wv = wcombG[:].rearrange("q (t h2 kk l) -> q h2 kk t l",
                                         t=4, h2=8, kk=8)
                for l in range(2):
                    nc.vector.tensor_tensor(
                        ptv[:, :, :, l, :], ghv[:, :, :, l, :],
                        wv[:, h, :, :, l].unsqueeze(-1)
                        .broadcast_to((128, 8, 4, 32)),
                        AL.mult)
                nc.vector.tensor_reduce(
                    OG[:].rearrange("q (t h2 c) -> q t h2 c", t=4, h2=8)[:, :, h, :],
                    pt[:].rearrange("q (t c s) -> q t c s", t=4, s=16),
                    AX.X, AL.add)

            # out-proj per tile; accumulate t2 = ps2 + bias + resid directly
            t2a = p2.tile([128, 4 * D], F32, tag="ln_t2a", name=f"lnt2a_{g}", bufs=1)
            for T in range(4):
                oT = p2.tile([128, D], F32R, tag="oTT")
                for k in range(2):
                    ps = ps_sh.tile([128, 128], F32, tag="s")
                    nc.tensor.matmul(ps[:], OG[:, T * 256 + k * 128:
                                                T * 256 + (k + 1) * 128],
                                     ident_f[:], is_transpose=True)
                    nc.vector.tensor_copy(oT[:, k * 128:(k + 1) * 128], ps[:])
                ps2 = ps_o.tile([128, D], F32, tag="o", name=f"ps2_{g}_{T}")
                for k in range(2):
                    nc.tensor.matmul(ps2[:], oT[:, k * 128:(k + 1) * 128],
                                     WoT[k][:], start=(k == 0), stop=(k == 1))
                nc.vector.tensor_tensor(t2a[:, T * D:(T + 1) * D], ps2[:],
                                        bout_full[:], AL.add)
                nc.vector.tensor_tensor(t2a[:, T * D:(T + 1) * D],
                                        t2a[:, T * D:(T + 1) * D],
                                        xr_t[T][:].bitcast(F32), AL.add)

            def layer_norm_grp(t2, gfull, befull, outdt, nm):
                """batched LN over 4 tiles: t2 [128, 4*D] pre-summed input."""
                t2v = t2[:].rearrange("q (t c) -> q t c", t=4)
                mu = p2.tile([128, 4], F32, tag="ln_mu", name=f"lnmu_{nm}_{g}")
                nc.vector.tensor_reduce(mu[:], t2v, AX.X, AL.add)
                nc.vector.tensor_scalar_mul(mu[:], mu[:], 1.0 / D)
                xc = p2.tile([128, 4 * D], F32, tag="ln_xc", name=f"lnxc_{nm}_{g}", bufs=1)
                xcv = xc[:].rearrange("q (t c) -> q t c", t=4)
                nc.vector.tensor_tensor(
                    xcv, t2v, mu[:].unsqueeze(-1).broadcast_to((128, 4, D)),
                    AL.subtract)
                sq = p2.tile([128, 4 * D], F32, tag="ln_sq", name=f"lnsq_{nm}_{g}", bufs=1)
                nc.scalar.activation(sq[:], xc[:], AF.Square)
                var = p2.tile([128, 4], F32, tag="ln_var", name=f"lnvar_{nm}_{g}")
                nc.vector.tensor_reduce(
                    var[:], sq[:].rearrange("q (t c) -> q t c", t=4), AX.X, AL.add)
                rs = p2.tile([128, 4], F32, tag="ln_rs", name=f"lnrs_{nm}_{g}")
                nc.vector.tensor_scalar(rs[:], var[:], 1.0 / D, 1e-5,
                                        AL.mult, AL.add)
                nc.scalar.activation(rs[:], rs[:], AF.Sqrt)
                nc.vector.reciprocal(rs[:], rs[:])
                z = p2.tile([128, 4 * D], F32, tag="ln_z", name=f"lnz_{nm}_{g}", bufs=1)
                zv = z[:].rearrange("q (t c) -> q t c", t=4)
                nc.vector.tensor_tensor(
                    zv, xcv, rs[:].unsqueeze(-1).broadcast_to((128, 4, D)),
                    AL.mult)
                nc.vector.tensor_tensor(
                    zv, zv, gfull[:].unsqueeze(1).broadcast_to((128, 4, D)),
                    AL.mult)
                o = p2.tile([128, 4 * D], outdt, tag=f"ln_o_{nm}",
                            name=f"lno_{nm}_{g}", bufs=1)
                nc.vector.tensor_tensor(
                    o[:].rearrange("q (t c) -> q t c", t=4), zv,
                    befull[:].unsqueeze(1).broadcast_to((128, 4, D)), AL.add)
                return o

            h1g = layer_norm_grp(t2a, g1_full, be1_full, F32, "1")
            h1T_g = [p2.tile([128, 512], F32R, tag=f"h1T{ck}",
                             name=f"h1T{g}_{ck}", bufs=1) for ck in range(2)]
            for T in range(4):
                for k in range(2):
                    ps = ps_sh.tile([128, 128], F32, tag="s")
                    nc.tensor.matmul(
                        ps[:], h1g[:, T * D + k * 128:T * D + (k + 1) * 128],
                        ident_f[:], is_transpose=True)
                    nc.vector.tensor_copy(h1T_g[k][:, T * 128:(T + 1) * 128],
                                          ps[:])

            rlu = [gp.tile([128, 512], F32R, tag=f"rlu{dfc}",
                           name=f"rlu{g}_{dfc}", bufs=1) for dfc in range(16)]
            for dfc in range(16):
                psf = ps_f.tile([128, 512], F32, tag="f")
                for ck in range(2):
                    nc.tensor.matmul(psf[:], W1T[ck][dfc][:], h1T_g[ck][:],
                                     start=(ck == 0), stop=(ck == 1))
                nc.scalar.activation(rlu[dfc][:], psf[:], AF.Relu,
                                     bias=b1t[:, dfc:dfc + 1], scale=1.0)
            t2b = p2.tile([128, 4 * D], F32, tag="ln_t2a", name=f"lnt2b_{g}", bufs=1)
            for T in range(4):
                pso = ps_o.tile([128, D], F32, tag="o", name=f"pso_{g}_{T}")
                for dfc in range(16):
                    nc.tensor.matmul(pso[:], rlu[dfc][:, T * 128:(T + 1) * 128],
                                     W2T[dfc][:], start=(dfc == 0),
                                     stop=(dfc == 15))
                nc.vector.tensor_tensor(t2b[:, T * D:(T + 1) * D], pso[:],
                                        b2_full[:], AL.add)
                nc.vector.tensor_tensor(t2b[:, T * D:(T + 1) * D],
                                        t2b[:, T * D:(T + 1) * D],
                                        h1g[:, T * D:(T + 1) * D], AL.add)
            o2g = layer_norm_grp(t2b, g2_full, be2_full, F32, "2")
            for T in range(4):
                r_loc = g * 4 + T
                nc.sync.dma_start(out[r_loc * W:(r_loc + 1) * W, :],
                                  o2g[:, T * D:(T + 1) * D])

        for cm in reversed(phase_cm):
            cm.__exit__(None, None, None)

    nc.compile()
    return nc


def kernel(**inputs):
    global _nc_cache
    if _nc_cache is None:
        _nc_cache = build_nc()
    nc = _nc_cache

    src = np.asarray(inputs["src"], np.float32)          # [2,128,128,256]
    f = lambda k: np.ascontiguousarray(np.asarray(inputs[k], np.float32))
    row = lambda k: f(k).reshape(1, -1)

    in_maps = []
    for c in range(8):
        b, j = c // 4, c % 4
        m = {
            "src_img": np.ascontiguousarray(src[b].reshape(H * W, D)),
            "src_q": np.ascontiguousarray(
                src[b, 32 * j:32 * (j + 1)].reshape(QROWS * W, D)),
            "rowbase": np.array([[32.0 * j]], np.float32),
            "w_off": f("W_off"), "b_off": row("b_off"),
            "w_attn": f("W_attn"), "b_attn": row("b_attn"),
            "w_val": f("W_val"), "b_val": row("b_val"),
            "w_out": f("W_out"), "b_out": row("b_out"),
            "w1": f("W1"), "b1": row("b1"),
            "w2": f("W2"), "b2": row("b2"),
            "g1": row("g1"), "be1": row("be1"),
            "g2": row("g2"), "be2": row("be2"),
        }
        in_maps.append(m)

    trace = bool(int(os.environ.get("K_TRACE", "0")))
    res = run_bass_kernel_spmd(nc, in_maps, core_ids=list(range(8)),
                               trace=trace)
    if trace:
        kernel.last_exec_time_ns = res.exec_time_ns
        kernel.last_trace = res.instructions_and_trace
    else:
        kernel.last_exec_time_ns = None
    quarters = [r["out"].reshape(QROWS, W, D) for r in
                (res.results[c] for c in range(8))]
    img0 = np.concatenate(quarters[0:4], axis=0)
    img1 = np.concatenate(quarters[4:8], axis=0)
    return np.stack([img0, img1], axis=0)
